# revision 4
# baseline (speedup 1.0000x reference)
"""Trainium2 Bass kernel for the maxtext-style quantized KV-cache update.

Computation (see problem reference):
  1. quantize the new decode-step K/V (per-(b,h) abs-max over D, rint)
  2. scatter-append at ar_cache_index into the stored (S,H,B,D) int8-valued
     cache + per-row scales
  3. return the fully dequantized caches  q * scale / 127.5  for K and V.

Strategy: tensor-parallel over heads — 16 heads -> 2 per NeuronCore, 8 cores.
The cache holds int8-valued floats (rint of randn*40); the host clips them
to int8 and streams that (4x less read traffic than f32) — the ~0.14% of
values with |q| > 127 are fixed up exactly on the host afterwards.  The
device casts int8 -> fp16, multiplies by the fp16 scales, and stores fp16
(~2.5e-4 relative error), which the host upcasts to f32.  HBM traffic per
core is ~19 MiB; the DMA system sustains ~425 GB/s aggregate, so the
schedule aims to keep the queues saturated end to end:

  - sync engine: issues all 6 int8 tile loads back-to-back at t=0, then
    issues each tile's store the moment its dequant multiply retires (the
    engine stalling on the mult semaphore is free — it has no other work —
    and it enqueues stores in exactly their readiness order).
  - scalar (Act) engine: aux + scale DMAs first, then int8->fp16 casts for
    4 of the 6 tiles, in half-tile chunks so the DVE multiply (and the
    store behind it) trails a chunk rather than the full 7.5us cast.
  - vector (DVE): the tiny replacement-row chain, casts for the other 2
    tiles, and all dequant multiplies (broadcast along the middle axis of
    the d-major layout keeps the 2-byte fast path).
  - gpsimd: the two replacement-row patch DMAs.

Layout: each core's 49,152 cache rows (K then V, row = one (s,h,b) D-vector,
64 rows per SBUF partition) are stored d-major *within* each partition:
element j of a partition = (d, c) = (j // 64, j % 64) of its 64-row slab.
The dequant multiply is then ct[p, d, c] *= scale16[p, c].  Scales are
pre-multiplied by 1/127.5 and pre-cast to fp16 on the host; the
replacement row is computed in exact f32 on the DVE, PE-transposed to
d-major, and patched into the output tile before its store.
"""

import os
import sys

if "/opt/trn_rl_repo" not in sys.path:
    sys.path.insert(0, "/opt/trn_rl_repo")

# The kernel executes through the axon/neuron PJRT backend; a leftover
# JAX_PLATFORMS=cpu (used for reference-side jax) would hide the NeuronCores.
if "jax" not in sys.modules:
    _jp = os.environ.get("JAX_PLATFORMS")
    if _jp is not None and "axon" not in _jp and "neuron" not in _jp:
        del os.environ["JAX_PLATFORMS"]

import numpy as np

B, H, D = 4, 16, 128
S_AR = 3072
NCORES = 8
HSH = H // NCORES            # heads per core
ROWS = S_AR * HSH * B        # rows per core-cache (24576)
F = 8192                     # SBUF tile free dim (elements)
CPS = F // D                 # rows (columns) per partition slab (64)
NT = 2 * ROWS * D // (128 * F)   # tiles over combined K+V rows (6)
TPC = NT // 2                # tiles per cache (3)
C_DEQ = float(np.float32(1.0 / 127.5))
MAX_INT8 = 127.5
MAGIC = 12582912.0           # 1.5 * 2**23: (x + MAGIC) - MAGIC == rint(x) in f32

TRACE = False                # test harness sets True to capture an NTFF profile
LAST_RESULT = None           # BassKernelResults of the most recent run

_PROG_CACHE = {}


def _build_program(s: int):
    import concourse.bacc as bacc
    import concourse.mybir as mybir
    from concourse.tile import TileContext

    f32 = mybir.dt.float32
    f16 = mybir.dt.float16
    op = mybir.AluOpType

    nc = bacc.Bacc("TRN2", target_bir_lowering=False, debug=False,
                   num_devices=NCORES)

    i8 = mybir.dt.int8
    NRR = HSH * B
    cin = nc.dram_tensor("cin", [NT, 128, F], i8, kind="ExternalInput")
    # all six tiles' scales, partition-major -> one DMA, one semaphore
    sc = nc.dram_tensor("sc", [128, NT * CPS], f16, kind="ExternalInput")
    # [ident(16x16) | nk rows ; nv rows (16x128)] -> one DMA, one K+V chain
    aux = nc.dram_tensor("aux", [2 * NRR, 2 * NRR + D], f32,
                         kind="ExternalInput")
    out = nc.dram_tensor("out", [NT, 128, F], f16, kind="ExternalOutput")

    # patch site of the replacement row for each cache: rows [8s, 8s+8) of
    # the cache's 24576 rows; 64-row slabs -> tile, partition, column
    NR = HSH * B                              # 8 rows per seq position
    patch = {}
    for i, nm in enumerate(("k", "v")):
        slab = i * (ROWS // CPS) + (s * NR) // CPS
        t_star, p_star = divmod(slab, 128)
        c0 = (s * NR) % CPS
        patch.setdefault(t_star, []).append((nm, p_star, c0))
    patch_tiles = sorted(patch)
    nonpatch = [t for t in range(NT) if t not in patch]
    order = patch_tiles + nonpatch

    with TileContext(nc) as tc:
        with tc.tile_pool(name="row", bufs=1) as rowpool, \
             tc.psum_pool(name="ps", bufs=2) as pspool, \
             tc.tile_pool(name="c8", bufs=NT) as c8pool, \
             tc.tile_pool(name="cp", bufs=NT) as cpool, \
             tc.tile_pool(name="sp", bufs=NT) as spool:
            # --- scalar queue: tiny aux + scales, in front of the casts
            NP = 2 * NR                          # 16 rows: K then V
            auxt = rowpool.tile([NP, NP + D], f32, tag="aux")
            nc.scalar.dma_start(auxt[:], aux[:])
            st_all = spool.tile([128, NT * CPS], f16, tag="st")
            nc.scalar.dma_start(st_all[:], sc[:])

            # --- sync queue: all six bulk loads, back-to-back at t=0
            c8s = []
            for pos, t in enumerate(order):
                c8 = c8pool.tile([128, F], i8, tag="c8")
                nc.sync.dma_start(c8[:], cin[t])
                c8s.append(c8)

            # --- dequantized replacement row (tiny, exact v1 math) on DVE;
            # transposed to (D, NR) via the idle PE so the patch DMA's
            # iteration order matches the d-major tile layout.
            idt = auxt[:, 0:NP]
            rt = auxt[:, NP:NP + D]
            sig = rowpool.tile([NP, 1], f32, tag="sig")
            nc.vector.tensor_reduce(sig[:], rt,
                                    axis=mybir.AxisListType.X,
                                    op=op.max, apply_absolute_value=True)
            rc = rowpool.tile([NP, 1], f32, tag="rc")
            nc.vector.reciprocal(rc[:], sig[:])
            rr = rowpool.tile([NP, 1], f32, tag="rr")
            nc.vector.tensor_scalar(rr[:], rc[:], MAX_INT8, None, op.mult)
            tt = rowpool.tile([NP, D], f32, tag="tt")
            nc.vector.tensor_scalar(tt[:], rt, rr[:], None, op.mult)
            qt = rowpool.tile([NP, D], f32, tag="qt")
            nc.vector.tensor_scalar(qt[:], tt[:], MAGIC, None, op.add)
            s2 = rowpool.tile([NP, 1], f32, tag="s2")
            nc.vector.tensor_scalar(s2[:], sig[:], C_DEQ, None, op.mult)
            dr32 = rowpool.tile([NP, D], f32, tag="dr32")
            nc.vector.tensor_scalar(dr32[:], qt[:], MAGIC, s2[:],
                                    op.subtract, op.mult)
            ps = pspool.tile([D, NP], f32, tag="ps")
            nc.tensor.transpose(ps[:], dr32[:], idt)
            dr = rowpool.tile([D, NP], f16, tag="dr")
            nc.vector.tensor_scalar(dr[:], ps[:], 1.0, None, op.mult)
            drow = {"k": dr[:, 0:NR], "v": dr[:, NR:NP]}

            # --- per-tile pipeline: fused cast+mult -> (patch) -> store.
            # One DVE tensor_tensor per tile reads the int8 tile directly
            # and multiplies by the fp16 scales (broadcast along the middle
            # axis of the d-major layout), writing the fp16 output tile —
            # no separate cast pass, and the Act engine stays free.
            # Stores are issued by the otherwise-idle sync engine; it
            # stalls on each tile's mult semaphore and thereby feeds the
            # sync HWDGE queue in exactly readiness order, directly behind
            # the loads — no DMA hole between the load and store phases.
            for pos, t in enumerate(order):
                c8 = c8s[pos]
                st = st_all[:, t * CPS:(t + 1) * CPS]
                ct = cpool.tile([128, F], f16, tag="ct")
                ct3 = ct[:].rearrange("p (d c) -> p d c", c=CPS)
                c83 = c8[:].rearrange("p (d c) -> p d c", c=CPS)
                stb = st.unsqueeze(1).broadcast_to((128, D, CPS))
                nc.vector.tensor_tensor(ct3, c83, stb, op.mult)
                for nm, p_star, c0 in patch.get(t, ()):
                    tgt = ct[p_star:p_star + 1].rearrange(
                        "p (d c) -> p d c", c=CPS)[:, :, c0:c0 + NR]
                    nc.gpsimd.dma_start(tgt, drow[nm])
                nc.sync.dma_start(out[t], ct[:])
    nc.compile()
    return nc


def _prog(s: int):
    if s not in _PROG_CACHE:
        _PROG_CACHE[s] = _build_program(s)
    return _PROG_CACHE[s]


def _to_dmajor(rows16):
    """(24576, 128) fp16 row-major -> (TPC, 128, F) d-major per 64-row slab."""
    a = rows16.reshape(TPC, 128, CPS, D)      # [t, p, c, d]
    return np.ascontiguousarray(a.transpose(0, 1, 3, 2)).reshape(TPC, 128, F)


def _from_dmajor(tiles16):
    """(TPC, 128, F) fp16 d-major -> (24576, 128) f32 row-major."""
    a = tiles16.reshape(TPC, 128, D, CPS).transpose(0, 1, 3, 2)
    return a.astype(np.float32).reshape(ROWS, D)


def kernel(key, value, cached_ar_key, cached_ar_value,
           cached_ar_key_scale, cached_ar_value_scale, ar_cache_index):
    global LAST_RESULT
    from concourse.bass_utils import run_bass_kernel_spmd

    key = np.asarray(key, dtype=np.float32)
    value = np.asarray(value, dtype=np.float32)
    cached_ar_key = np.asarray(cached_ar_key, dtype=np.float32)
    cached_ar_value = np.asarray(cached_ar_value, dtype=np.float32)
    cached_ar_key_scale = np.asarray(cached_ar_key_scale, dtype=np.float32)
    cached_ar_value_scale = np.asarray(cached_ar_value_scale, dtype=np.float32)
    s = int(ar_cache_index)

    nc = _prog(s)

    # int8-valued cache entries: stream the int8 clip through the device,
    # fix up the rare clipped outliers (|q| > 127) exactly on the host
    k8 = np.clip(cached_ar_key, -128, 127).astype(np.int8)
    v8 = np.clip(cached_ar_value, -128, 127).astype(np.int8)
    key_t = np.ascontiguousarray(key[:, 0].transpose(1, 0, 2))      # (H,B,D)
    val_t = np.ascontiguousarray(value[:, 0].transpose(1, 0, 2))

    in_maps = []
    for i in range(NCORES):
        h0 = i * HSH
        hs = slice(h0, h0 + HSH)
        cin = np.empty((NT, 128, F), np.int8)
        cin[:TPC] = _to_dmajor(k8[:, hs].reshape(ROWS, D))
        cin[TPC:] = _to_dmajor(v8[:, hs].reshape(ROWS, D))
        scf = np.empty((NT, 128, CPS), np.float32)
        scf[:TPC] = cached_ar_key_scale[:, hs].reshape(TPC, 128, CPS)
        scf[TPC:] = cached_ar_value_scale[:, hs].reshape(TPC, 128, CPS)
        sc16 = (scf * np.float32(C_DEQ)).astype(np.float16)
        npp = 2 * HSH * B
        aux = np.empty((npp, npp + D), np.float32)
        aux[:, :npp] = np.eye(npp, dtype=np.float32)
        aux[:HSH * B, npp:] = key_t[hs].reshape(HSH * B, D)
        aux[HSH * B:, npp:] = val_t[hs].reshape(HSH * B, D)
        in_maps.append({
            "cin": cin,
            "sc": np.ascontiguousarray(sc16.transpose(1, 0, 2)).reshape(
                128, NT * CPS),
            "aux": aux,
        })

    res = run_bass_kernel_spmd(nc, in_maps, list(range(NCORES)), trace=TRACE)
    LAST_RESULT = res

    k_out = np.empty((S_AR, H, B, D), np.float32)
    v_out = np.empty((S_AR, H, B, D), np.float32)
    for i, r in enumerate(res.results):
        h0 = i * HSH
        o = np.asarray(r["out"])
        k_out[:, h0:h0 + HSH] = _from_dmajor(o[:TPC]).reshape(S_AR, HSH, B, D)
        v_out[:, h0:h0 + HSH] = _from_dmajor(o[TPC:]).reshape(S_AR, HSH, B, D)

    # exact host fixup of int8-clipped outliers (row s comes from the new
    # decode step on device, so its stale cache values are excluded)
    for cache, scale, outa in ((cached_ar_key, cached_ar_key_scale, k_out),
                               (cached_ar_value, cached_ar_value_scale, v_out)):
        mask = np.abs(cache) > 127
        mask[s] = False
        idx = np.nonzero(mask)
        outa[idx] = cache[idx] * (scale[idx[0], idx[1], idx[2], 0]
                                  * np.float32(C_DEQ))
    return k_out, v_out
